# revision 30
# baseline (speedup 1.0000x reference)
"""3-layer GCN (GCNConv x3 + leaky_relu + first-node-per-graph readout) on
8 Trainium2 NeuronCores via Bass/Tile.

Strategy (src-partitioned edges + quartered ReduceScatter):
  - Destination nodes are BIN-PACKED into windows of <=128 nodes such that
    every (src-core, window) cell holds at most KW*128 = 256 edges; windows
    are assigned contiguously to 8 cores. Each core owns the feature rows
    (x-tilde / h1-tilde tables in local DRAM) of its windows' nodes, and all
    edges whose SOURCE it owns -- gathers only ever touch local tables, so
    no feature AllGather exists.
  - GCN normalization is factored: out = dis_d * segsum_dst((dis*h)[src]) @ W
    + b with dis = deg^-1/2.
  - Per edge-chunk of 128, one PE matmul psum[c, d] += g[e, c]^T @ S[e, d]
    with S a host-precomputed fp8 one-hot (edge -> dst slot) matrix builds
    CHANNEL-MAJOR partial aggregations for all windows.
  - Windows are processed in 4 "quarters" (quarter-major, then dst-core,
    then window); after each quarter a ReduceScatter (op=add) reduces that
    quarter's partials, overlapping collectives with later aggregation.
  - Layer 3 only needs ~1.6k edges (dst == first node of a graph): each core
    aggregates its local z-tilde rows into per-graph partials; a tiny
    AllGather + on-chip sum finishes the readout; core 0's output is used.

kernel(**inputs) takes the full unsharded inputs and returns the full
[n_graphs, 32] float32 output.
"""

import sys

sys.path.insert(0, "/opt/trn_rl_repo")

import numpy as np

import concourse.bacc as bacc
import concourse.mybir as mybir
import concourse.tile as tile
from concourse.bass_utils import run_bass_kernel_spmd

F32 = mybir.dt.float32
BF16 = mybir.dt.bfloat16
FP8 = mybir.dt.float8e4
I16 = mybir.dt.int16

N_CORES = 8
C0, C1, C2, C3 = 128, 256, 256, 32
ZPAD = 64
QW = 4  # reduce-scatter quarters

OUT_NAMES = ["out"]


def _pack_gather_idx(idx, n_slots):
    """int32 row indices -> dma_gather int16 layout [128, n_slots//16]."""
    assert n_slots % 16 == 0
    a = np.zeros(n_slots, np.int16)
    a[: len(idx)] = idx.astype(np.int16)
    a = a.reshape(n_slots // 16, 16).T  # [16, cols]
    return np.tile(a, (8, 1))  # [128, cols]


def _bin_pack(dst, N, KW):
    """Pack dst nodes into windows of <=128 nodes with per-src-core edge
    quota KW*128. Returns win_of[n], slot_of[n], n_windows."""
    owner_guess = None  # packing is independent of src owner? no:
    # per-core in-edge counts c[i, n]: needs src owners, passed via closure
    raise NotImplementedError


def host_prep(x, src, dst, batch, W1, b1, W2, b2, W3, b3, n_graphs):
    import ml_dtypes

    N = x.shape[0]
    G = int(n_graphs)
    E = len(src)
    KW = 2
    CAP = KW * 128

    deg = np.bincount(dst, minlength=N).astype(np.float32)
    dis = np.where(deg > 0, 1.0 / np.sqrt(deg), 0.0).astype(np.float32)

    first = np.full(G, N, np.int64)
    np.minimum.at(first, batch.astype(np.int64), np.arange(N))

    # ---------- destination bin-packing ----------
    # src ownership must be decided first; we keep srcs owned by the core
    # that owns them as DESTINATIONS (same node partition), so pack first
    # using per-prospective-core counts. To break the circularity, packing
    # quota uses per-core in-edge counts computed from the FINAL owner of
    # each src, which is itself the packing result. Instead we use a
    # 2-pass scheme: pass 1 packs by contiguous-owner counts, pass 2 uses
    # those owners. In practice edges are uniform; one pass with the
    # contiguous partition's owner estimate is enough to keep KW=2.
    NPC0 = N // N_CORES
    owner0 = np.minimum(src // NPC0, N_CORES - 1)
    cin = np.zeros((N_CORES, N), np.int32)
    np.add.at(cin, (owner0, dst), 1)

    # greedy pack: descending total degree
    order = np.argsort(-deg, kind="stable")
    win_of = np.full(N, -1, np.int64)
    slot_of = np.full(N, -1, np.int64)
    wins = []  # per window: [count, percore_vec]
    open_w = []  # indices of open windows
    for n in order:
        c = cin[:, n]
        placed = False
        for wi in open_w:
            cnt, vec = wins[wi]
            if cnt < 128 and np.all(vec + c <= CAP):
                win_of[n] = wi
                slot_of[n] = cnt
                wins[wi][0] += 1
                vec += c
                placed = True
                if wins[wi][0] == 128:
                    open_w.remove(wi)
                break
        if not placed:
            wi = len(wins)
            wins.append([1, c.astype(np.int64).copy()])
            win_of[n] = wi
            slot_of[n] = 0
            if 128 > 1:
                open_w.append(wi)
            if len(open_w) > 12:
                open_w.pop(0)  # cap scan cost; old windows stay partial

    nwin = len(wins)
    wpc = (nwin + N_CORES - 1) // N_CORES
    # quarter boundaries over local windows
    qb = [round(k * wpc / QW) for k in range(QW + 1)]
    qsz = [qb[k + 1] - qb[k] for k in range(QW)]
    NPADc = wpc * 128

    core_of = np.minimum(win_of // wpc, N_CORES - 1)
    vloc = win_of - core_of * wpc  # local window
    lrow = vloc * 128 + slot_of  # local node row (window-major)

    def rmap(lr):
        return (lr % 128) * wpc + lr // 128

    # ---------- edges ----------
    owner_s = core_of[src]
    # processing order of local windows: quarter-major, then dst core, then v
    worder = []  # (core j, local v) -> position
    pos_of = np.full((N_CORES, wpc), -1, np.int64)
    p = 0
    for k in range(QW):
        for j in range(N_CORES):
            for v in range(qb[k], qb[k + 1]):
                pos_of[j, v] = p
                worder.append((j, v))
                p += 1
    NWIN_T = p  # == 8 * wpc
    NCH = NWIN_T * KW
    NSLOT = NCH * 128

    epos = pos_of[core_of[dst], vloc[dst]]  # per edge: window position

    # ---------- layer-3 edges ----------
    is_first = np.zeros(N, bool)
    is_first[first] = True
    gid_of = np.full(N, -1, np.int64)
    gid_of[first] = np.arange(G)
    e3 = np.nonzero(is_first[dst])[0]
    e3_owner = owner_s[e3]
    cnt3 = np.bincount(e3_owner, minlength=N_CORES)
    P3 = max(1, int(np.ceil(cnt3.max() / 128)))
    NS3 = P3 * 128

    eyeb = np.eye(128, dtype=ml_dtypes.bfloat16)
    eyef = np.eye(128, dtype=np.float32)
    b1c = np.ascontiguousarray(b1.reshape(2, 128).T)
    b2c = np.ascontiguousarray(b2.reshape(2, 128).T)
    b3p = np.zeros(ZPAD, np.float32)
    b3p[:C3] = b3
    b3bc = np.tile(b3p[None, :], (128, 1))
    disf = np.zeros((128, 1), np.float32)
    disf[:G, 0] = dis[first]

    w1b = W1.astype(ml_dtypes.bfloat16)
    w2r = np.zeros((128, 4 * 128), ml_dtypes.bfloat16)
    for kh in range(2):
        for mh in range(2):
            w2r[:, (kh * 2 + mh) * 128 : (kh * 2 + mh + 1) * 128] = W2[
                kh * 128 : (kh + 1) * 128, mh * 128 : (mh + 1) * 128
            ].astype(ml_dtypes.bfloat16)
    w3r = np.zeros((128, 2 * C3), ml_dtypes.bfloat16)
    for kh in range(2):
        w3r[:, kh * C3 : (kh + 1) * C3] = W3[kh * 128 : (kh + 1) * 128, :].astype(
            ml_dtypes.bfloat16
        )

    node_at = np.full((128, wpc, N_CORES), -1, np.int64)  # slot, v, core -> n
    node_at[slot_of, vloc, core_of] = np.arange(N)

    in_maps = []
    for i in range(N_CORES):
        sel = node_at[:, :, i]  # [128, wpc]
        valid = sel >= 0
        xs = np.zeros((128, wpc, C0), np.float32)
        xs[valid] = x[sel[valid]]
        dwin = np.zeros((128, wpc), np.float32)
        dwin[valid] = dis[sel[valid]]
        disw = np.ascontiguousarray(dwin)
        dl = np.zeros(NPADc, np.float32)
        dl[(vloc[sel[valid]] * 128 + slot_of[sel[valid]])] = dis[sel[valid]]
        disbc = np.tile(dl.astype(ml_dtypes.bfloat16)[None, :], (128, 1))

        ei = np.nonzero(owner_s == i)[0]
        ei = ei[np.argsort(epos[ei], kind="stable")]
        cntw = np.bincount(epos[ei], minlength=NWIN_T)
        assert cntw.max() <= CAP, f"bin packing failed: {cntw.max()}"
        ptr = np.concatenate([[0], np.cumsum(cntw)])
        eslot = np.zeros(len(ei), np.int64)
        for wp in range(NWIN_T):
            ee = np.arange(ptr[wp], ptr[wp + 1])
            eslot[ee] = wp * CAP + np.arange(len(ee))
        idx_flat = np.zeros(NSLOT, np.int64)
        idx_flat[eslot] = rmap(lrow[src[ei]])
        S_host = np.zeros((128, NCH, 128), ml_dtypes.float8_e4m3)
        S_host[eslot % 128, eslot // 128, slot_of[dst[ei]]] = 1.0
        idx_l = _pack_gather_idx(idx_flat, NSLOT)

        ee3 = e3[e3_owner == i]
        idx3 = _pack_gather_idx(rmap(lrow[src[ee3]]), NS3)
        S3_host = np.zeros((128, P3, 128), ml_dtypes.float8_e4m3)
        l3 = np.arange(len(ee3))
        S3_host[l3 % 128, l3 // 128, gid_of[dst[ee3]]] = 1.0

        in_maps.append(
            {
                "xs": xs,
                "idx": idx_l,
                "s": np.ascontiguousarray(S_host),
                "idx3": idx3,
                "s3": np.ascontiguousarray(S3_host),
                "disw": disw,
                "disbc": disbc,
                "disf": disf,
                "b1c": b1c,
                "b2c": b2c,
                "b3bc": b3bc,
                "w1": np.asarray(w1b),
                "w2r": np.asarray(w2r),
                "w3r": np.asarray(w3r),
                "eyeb": np.asarray(eyeb),
                "eyef": eyef,
            }
        )

    meta = dict(
        N=N, G=G, KW=KW, NCH=NCH, NSLOT=NSLOT, P3=P3, wpc=wpc, qb=tuple(qb)
    )
    return in_maps, meta


# ---------------------------------------------------------------------------
# Device program
# ---------------------------------------------------------------------------


def build_program(meta, compile_=True, repeat=1):
    KW, NCH, NSLOT, P3 = meta["KW"], meta["NCH"], meta["NSLOT"], meta["P3"]
    wpc, qb = meta["wpc"], list(meta["qb"])
    qsz = [qb[k + 1] - qb[k] for k in range(QW)]
    NPADc = wpc * 128
    NS3 = P3 * 128
    CAP = KW * 128

    nc = bacc.Bacc(
        "TRN2", target_bir_lowering=False, debug=False, num_devices=N_CORES
    )
    dp = nc.declare_dram_parameter
    xs_d = dp("xs", [128, wpc, C0], F32, isOutput=False)
    idx_d = dp("idx", [128, NSLOT // 16], I16, isOutput=False)
    s_d = dp("s", [128, NCH, 128], FP8, isOutput=False)
    idx3_d = dp("idx3", [128, NS3 // 16], I16, isOutput=False)
    s3_d = dp("s3", [128, P3, 128], FP8, isOutput=False)
    disw_d = dp("disw", [128, wpc], F32, isOutput=False)
    disbc_d = dp("disbc", [128, NPADc], BF16, isOutput=False)
    disf_d = dp("disf", [128, 1], F32, isOutput=False)
    b1c_d = dp("b1c", [128, 2], F32, isOutput=False)
    b2c_d = dp("b2c", [128, 2], F32, isOutput=False)
    b3bc_d = dp("b3bc", [128, ZPAD], F32, isOutput=False)
    w1_d = dp("w1", [128, C1], BF16, isOutput=False)
    w2r_d = dp("w2r", [128, 4 * 128], BF16, isOutput=False)
    w3r_d = dp("w3r", [128, 2 * C3], BF16, isOutput=False)
    eyeb_d = dp("eyeb", [128, 128], BF16, isOutput=False)
    eyef_d = dp("eyef", [128, 128], F32, isOutput=False)
    out_d = dp("out", [128, ZPAD], F32, isOutput=True)

    rg = [list(range(N_CORES))]
    AL = mybir.AluOpType
    ACT = mybir.ActivationFunctionType

    with tile.TileContext(nc) as tc:
        with (
            tc.tile_pool(name="const", bufs=1) as cpool,
            tc.tile_pool(name="work", bufs=4) as pool,
            tc.tile_pool(name="slab", bufs=1) as bigpool,
            tc.tile_pool(name="pslab", bufs=2) as spool,
            tc.tile_pool(name="gath", bufs=6) as gpool,
            tc.tile_pool(name="psA", bufs=2, space="PSUM") as psA,
            tc.tile_pool(name="psB", bufs=3, space="PSUM") as psB,
            tc.tile_pool(name="psD", bufs=2, space="PSUM") as psD,
            tc.tile_pool(name="psT", bufs=1, space="PSUM") as psT,
            tc.tile_pool(name="dram", bufs=1, space="DRAM") as dram,
        ):
            def cload(name, shape, dt, src, eng=None):
                t = cpool.tile(shape, dt, tag=name, name=name + "_sb")
                (eng or nc.sync).dma_start(out=t[:], in_=src)
                return t

            idx_sb = cload("idx", [128, NSLOT // 16], I16, idx_d[:, :])
            # S loaded in quarter chunks so early matmuls start sooner
            s_sb = cpool.tile([128, NCH, 128], FP8, tag="s", name="s_sb")
            SCH = [qb[k] * N_CORES * KW for k in range(QW + 1)]
            for k in range(QW):
                nc.sync.dma_start(
                    out=s_sb[:, SCH[k] : SCH[k + 1], :],
                    in_=s_d[:, SCH[k] : SCH[k + 1], :],
                )
            idx3_sb = cload("idx3", [128, NS3 // 16], I16, idx3_d[:, :])
            s3_sb = cload("s3", [128, P3, 128], FP8, s3_d[:, :, :])
            disw = cload("disw", [128, wpc], F32, disw_d[:, :])
            disbc = cload("disbc", [128, NPADc], BF16, disbc_d[:, :])
            disf = cload("disf", [128, 1], F32, disf_d[:, :])
            b1c = cload("b1c", [128, 2], F32, b1c_d[:, :])
            b2c = cload("b2c", [128, 2], F32, b2c_d[:, :])
            b3bc = cload("b3bc", [128, ZPAD], F32, b3bc_d[:, :])
            w1 = cload("w1", [128, C1], BF16, w1_d[:, :])
            w2r = cload("w2r", [128, 4 * 128], BF16, w2r_d[:, :])
            w3r = cload("w3r", [128, 2 * C3], BF16, w3r_d[:, :])
            eyeb = cload("eyeb", [128, 128], BF16, eyeb_d[:, :])
            eyef = cload("eyef", [128, 128], F32, eyef_d[:, :])

            for _rep in range(repeat):
                xt_loc = dram.tile([NPADc, C0], BF16)
                h1t_loc = dram.tile([NPADc, C1], BF16)
                zt_loc = dram.tile([NPADc, 128], BF16)
                p1q = [
                    dram.tile(
                        [N_CORES, C0, qsz[k] * 128], BF16, name=f"p1q{k}"
                    )
                    for k in range(QW)
                ]
                p2q = [
                    dram.tile(
                        [N_CORES, C1, qsz[k] * 128], BF16, name=f"p2q{k}"
                    )
                    for k in range(QW)
                ]
                a1q = [
                    dram.tile([C0, qsz[k] * 128], BF16, name=f"a1q{k}")
                    for k in range(QW)
                ]
                a2q = [
                    dram.tile([C1, qsz[k] * 128], BF16, name=f"a2q{k}")
                    for k in range(QW)
                ]
                ar_in = dram.tile([128, ZPAD], F32)
                ag3_d = dram.tile([N_CORES, 128, ZPAD], F32, addr_space="Shared")

                # ---- stage X: x-tilde table ----
                xsl = bigpool.tile([128, wpc, C0], F32, tag="xsl")
                nc.scalar.dma_start(out=xsl[:], in_=xs_d[:, :, :])
                xts = bigpool.tile([128, wpc, C0], BF16, tag="xts")
                for v in range(wpc):
                    nc.vector.tensor_scalar(
                        xts[:, v, :], xsl[:, v, :], disw[:, v : v + 1], None,
                        AL.mult,
                    )
                nc.sync.dma_start(out=xt_loc[:, :], in_=xts[:])

                # ---- partial aggregation (quarter-major) ----
                def agg_layer(table, Cin, pq, aq, tag):
                    CALL = 8
                    ncalls = (NCH + CALL - 1) // CALL
                    gtiles = [None] * ncalls
                    issued = 0

                    def ensure(call_i):
                        nonlocal issued
                        while issued <= call_i:
                            m = issued
                            cs = min(CALL, NCH - m * CALL)
                            g = gpool.tile(
                                [128, CALL, Cin], BF16, tag=tag, name=tag + "g"
                            )
                            nc.gpsimd.dma_gather(
                                g[:, 0:cs, :],
                                table[:, :],
                                idx_sb[:, m * CALL * 8 : (m * CALL + cs) * 8],
                                num_idxs=cs * 128,
                                num_idxs_reg=cs * 128,
                                elem_size=Cin,
                            )
                            gtiles[m] = g
                            issued += 1

                    nh = Cin // 128
                    wp = 0
                    for k in range(QW):
                        for j in range(N_CORES):
                            pcs = [
                                spool.tile(
                                    [128, qsz[k] * 128], BF16,
                                    tag=f"{tag}s{h}", name=f"pc_{tag}{h}",
                                )
                                for h in range(nh)
                            ]
                            for t in range(qsz[k]):
                                ensure((wp * KW) // CALL)
                                ensure(((wp + 1) * KW - 1) // CALL)
                                ps = (psA if Cin == C0 else psB).tile(
                                    [128, Cin], F32, tag="agg"
                                )
                                for h in range(nh):
                                    for jj in range(KW):
                                        kk = wp * KW + jj
                                        g = gtiles[kk // CALL]
                                        nc.tensor.matmul(
                                            ps[:, h * 128 : (h + 1) * 128],
                                            lhsT=g[
                                                :, kk % CALL,
                                                h * 128 : (h + 1) * 128,
                                            ],
                                            rhs=s_sb[:, kk, :],
                                            start=(jj == 0),
                                            stop=(jj == KW - 1),
                                        )
                                ws = t * 128
                                for h in range(nh):
                                    if Cin == C0:
                                        nc.scalar.activation(
                                            pcs[h][:, ws : ws + 128],
                                            ps[:, h * 128 : (h + 1) * 128],
                                            ACT.Copy,
                                        )
                                    else:
                                        nc.vector.tensor_copy(
                                            pcs[h][:, ws : ws + 128],
                                            ps[:, h * 128 : (h + 1) * 128],
                                        )
                                wp += 1
                            for h in range(nh):
                                nc.sync.dma_start(
                                    out=pq[k][j, h * 128 : (h + 1) * 128, :],
                                    in_=pcs[h][:],
                                )
                        nc.gpsimd.collective_compute(
                            "ReduceScatter", AL.add, replica_groups=rg,
                            ins=[pq[k].opt()], outs=[aq[k].opt()],
                        )

                # ---- L1 ----
                agg_layer(xt_loc, C0, p1q, a1q, "g1")

                # ---- dense 1 ----
                h1slab = bigpool.tile([128, wpc, C1], BF16, tag="h1slab")
                for k in range(QW):
                    a1 = pool.tile(
                        [128, qsz[k] * 128], BF16, tag="a1", name="a1t"
                    )
                    nc.sync.dma_start(out=a1[:], in_=a1q[k][:, :])
                    for t in range(qsz[k]):
                        v = qb[k] + t
                        hp = psD.tile([128, C1], F32, tag="dense")
                        for mh in range(2):
                            nc.tensor.matmul(
                                hp[:, mh * 128 : (mh + 1) * 128],
                                lhsT=w1[:, mh * 128 : (mh + 1) * 128],
                                rhs=a1[:, t * 128 : (t + 1) * 128],
                                start=True,
                                stop=True,
                            )
                        dv = disbc[:, v * 128 : (v + 1) * 128]
                        for mh in range(2):
                            sl = slice(mh * 128, (mh + 1) * 128)
                            q = pool.tile([128, 128], F32, tag="q")
                            nc.vector.tensor_tensor(
                                q[:], hp[:, sl], dv, op=AL.mult
                            )
                            nc.vector.tensor_scalar(
                                q[:], q[:], b1c[:, mh : mh + 1], None, AL.add
                            )
                            vv = pool.tile([128, 128], F32, tag="v")
                            nc.scalar.activation(
                                vv[:], q[:], ACT.Copy, scale=0.01
                            )
                            nc.vector.tensor_tensor(q[:], q[:], vv[:], op=AL.max)
                            th = pool.tile([128, 128], BF16, tag="th")
                            nc.vector.tensor_tensor(th[:], q[:], dv, op=AL.mult)
                            tp = psT.tile([128, C1], BF16, tag="tr")
                            nc.tensor.transpose(tp[:, sl], th[:], eyeb[:])
                            nc.vector.tensor_copy(h1slab[:, v, sl], tp[:, sl])
                nc.sync.dma_start(out=h1t_loc[:, :], in_=h1slab[:])

                # ---- L2 ----
                agg_layer(h1t_loc, C1, p2q, a2q, "g2")

                # ---- dense 2 + z ----
                zslab = bigpool.tile([128, wpc, 128], BF16, tag="zslab")
                nc.vector.memset(zslab[:], 0.0)
                for k in range(QW):
                    a2 = [
                        pool.tile(
                            [128, qsz[k] * 128], BF16, tag=f"a2_{kh}",
                            name=f"a2t{kh}",
                        )
                        for kh in range(2)
                    ]
                    for kh in range(2):
                        nc.sync.dma_start(
                            out=a2[kh][:],
                            in_=a2q[k][kh * 128 : (kh + 1) * 128, :],
                        )
                    for t in range(qsz[k]):
                        v = qb[k] + t
                        hp = psD.tile([128, C2], F32, tag="dense")
                        for mh in range(2):
                            for kh in range(2):
                                nc.tensor.matmul(
                                    hp[:, mh * 128 : (mh + 1) * 128],
                                    lhsT=w2r[
                                        :,
                                        (kh * 2 + mh) * 128 : (kh * 2 + mh + 1)
                                        * 128,
                                    ],
                                    rhs=a2[kh][:, t * 128 : (t + 1) * 128],
                                    start=(kh == 0),
                                    stop=(kh == 1),
                                )
                        dv = disbc[:, v * 128 : (v + 1) * 128]
                        h2s = pool.tile([128, C2], BF16, tag="h2s")
                        for mh in range(2):
                            sl = slice(mh * 128, (mh + 1) * 128)
                            q = pool.tile([128, 128], F32, tag="q")
                            nc.vector.tensor_tensor(
                                q[:], hp[:, sl], dv, op=AL.mult
                            )
                            nc.vector.tensor_scalar(
                                q[:], q[:], b2c[:, mh : mh + 1], None, AL.add
                            )
                            vv = pool.tile([128, 128], F32, tag="v")
                            nc.scalar.activation(
                                vv[:], q[:], ACT.Copy, scale=0.01
                            )
                            nc.vector.tensor_tensor(q[:], q[:], vv[:], op=AL.max)
                            nc.vector.tensor_copy(h2s[:, sl], q[:])
                        zp = psD.tile([128, C2], F32, tag="dense")
                        for kh in range(2):
                            nc.tensor.matmul(
                                zp[0:C3, 0:128],
                                lhsT=w3r[:, kh * C3 : (kh + 1) * C3],
                                rhs=h2s[:, kh * 128 : (kh + 1) * 128],
                                start=(kh == 0),
                                stop=(kh == 1),
                            )
                        zs = pool.tile([128, 128], F32, tag="zs")
                        nc.vector.tensor_tensor(
                            zs[0:C3, :], zp[0:C3, 0:128], dv[0:C3, :], op=AL.mult
                        )
                        ztp = psD.tile([128, C2], F32, tag="dense")
                        nc.tensor.transpose(
                            ztp[:, 0:C3], zs[0:C3, :], eyef[0:C3, 0:C3]
                        )
                        nc.vector.tensor_copy(zslab[:, v, 0:C3], ztp[:, 0:C3])
                nc.sync.dma_start(out=zt_loc[:, :], in_=zslab[:])

                # ---- L3 readout ----
                g3 = gpool.tile([128, P3, 128], BF16, tag="g3")
                nc.gpsimd.dma_gather(
                    g3[:, :, :],
                    zt_loc[:, :],
                    idx3_sb[:, :],
                    num_idxs=NS3,
                    num_idxs_reg=NS3,
                    elem_size=128,
                )
                o3 = psD.tile([128, C2], F32, tag="dense")
                for pp in range(P3):
                    nc.tensor.matmul(
                        o3[:, 0:128],
                        lhsT=s3_sb[:, pp, :],
                        rhs=g3[:, pp, :],
                        start=(pp == 0),
                        stop=(pp == P3 - 1),
                    )
                o3s = pool.tile([128, ZPAD], F32, tag="o3s")
                nc.vector.tensor_copy(o3s[:], o3[:, 0:ZPAD])
                nc.sync.dma_start(out=ar_in[:, :], in_=o3s[:])
                nc.gpsimd.collective_compute(
                    "AllGather", AL.bypass, replica_groups=rg,
                    ins=[ar_in.opt()], outs=[ag3_d.opt()],
                )
                acc = pool.tile([128, ZPAD], F32, tag="acc")
                nc.sync.dma_start(out=acc[:], in_=ag3_d[0, :, :])
                for j in range(1, N_CORES):
                    t = pool.tile([128, ZPAD], F32, tag="accj", name="accj")
                    nc.sync.dma_start(out=t[:], in_=ag3_d[j, :, :])
                    nc.vector.tensor_tensor(acc[:], acc[:], t[:], op=AL.add)
                nc.vector.tensor_scalar(
                    acc[:], acc[:], disf[:, 0:1], None, AL.mult
                )
                nc.vector.tensor_tensor(acc[:], acc[:], b3bc[:], op=AL.add)
                nc.sync.dma_start(out=out_d[:, :], in_=acc[:])

    if compile_:
        nc.compile()
    return nc


# ---------------------------------------------------------------------------
# Entry point
# ---------------------------------------------------------------------------

_cache = {}


def _prepare(inputs):
    in_maps, meta = host_prep(**inputs)
    key = (meta["KW"], meta["NCH"], meta["P3"], meta["wpc"], meta["qb"])
    if key not in _cache:
        _cache[key] = build_program(meta)
    return _cache[key], in_maps, meta


def assemble_output(results, meta):
    G = meta["G"]
    return np.ascontiguousarray(results[0]["out"][:G, :C3])


def kernel(**inputs):
    nc, in_maps, meta = _prepare(inputs)
    res = run_bass_kernel_spmd(nc, in_maps, core_ids=list(range(N_CORES)))
    return assemble_output(res.results, meta)


if __name__ == "__main__":
    rng = np.random.default_rng(0)
    N, E, G = 20000, 320000, 100
    inputs = dict(
        x=rng.standard_normal((N, 128), dtype=np.float32),
        src=rng.integers(0, N, E).astype(np.int32),
        dst=rng.integers(0, N, E).astype(np.int32),
        batch=(np.arange(N) // (N // G)).astype(np.int32),
        W1=rng.standard_normal((128, 256), dtype=np.float32) / 11.3,
        b1=rng.standard_normal(256).astype(np.float32) * 0.01,
        W2=rng.standard_normal((256, 256), dtype=np.float32) / 16.0,
        b2=rng.standard_normal(256).astype(np.float32) * 0.01,
        W3=rng.standard_normal((256, 32), dtype=np.float32) / 16.0,
        b3=rng.standard_normal(32).astype(np.float32) * 0.01,
        n_graphs=G,
    )
    out = kernel(**inputs)
    print("out", out.shape, out.dtype, float(np.abs(out).max()))


# revision 45
# speedup vs baseline: 1.8967x; 1.8967x over previous
"""3-layer GCN (GCNConv x3 + leaky_relu + first-node-per-graph readout) on
8 Trainium2 NeuronCores via Bass/Tile.

Strategy (src-partitioned edges + quartered ReduceScatter):
  - Destination nodes are BIN-PACKED into windows of <=128 nodes such that
    every (src-core, window) cell holds at most KW*128 = 256 edges; windows
    are assigned contiguously to 8 cores. Each core owns the feature rows
    (x-tilde / h1-tilde tables in local DRAM) of its windows' nodes, and all
    edges whose SOURCE it owns -- gathers only ever touch local tables, so
    no feature AllGather exists.
  - GCN normalization is factored: out = dis_d * segsum_dst((dis*h)[src]) @ W
    + b with dis = deg^-1/2.
  - Per edge-chunk of 128, one PE matmul psum[c, d] += g[e, c]^T @ S[e, d]
    with S a host-precomputed fp8 one-hot (edge -> dst slot) matrix builds
    CHANNEL-MAJOR partial aggregations for all windows.
  - Windows are processed in 4 "quarters" (quarter-major, then dst-core,
    then window); after each quarter a ReduceScatter (op=add) reduces that
    quarter's partials, overlapping collectives with later aggregation.
  - Layer 3 only needs ~1.6k edges (dst == first node of a graph): each core
    aggregates its local z-tilde rows into per-graph partials; a tiny
    AllGather + on-chip sum finishes the readout; core 0's output is used.

kernel(**inputs) takes the full unsharded inputs and returns the full
[n_graphs, 32] float32 output.
"""

import sys

sys.path.insert(0, "/opt/trn_rl_repo")

import numpy as np

import concourse.bacc as bacc
import concourse.mybir as mybir
import concourse.tile as tile
from concourse.bass_utils import run_bass_kernel_spmd

F32 = mybir.dt.float32
BF16 = mybir.dt.bfloat16
FP8 = mybir.dt.float8e4
I16 = mybir.dt.int16

N_CORES = 8
C0, C1, C2, C3 = 128, 256, 256, 32
ZPAD = 64
QW = 2  # reduce-scatter phases

OUT_NAMES = ["out"]


def _pack_gather_idx(idx, n_slots):
    """int32 row indices -> dma_gather int16 layout [128, n_slots//16]."""
    assert n_slots % 16 == 0
    a = np.zeros(n_slots, np.int16)
    a[: len(idx)] = idx.astype(np.int16)
    a = a.reshape(n_slots // 16, 16).T  # [16, cols]
    return np.tile(a, (8, 1))  # [128, cols]


def _bin_pack(dst, N, KW):
    """Pack dst nodes into windows of <=128 nodes with per-src-core edge
    quota KW*128. Returns win_of[n], slot_of[n], n_windows."""
    owner_guess = None  # packing is independent of src owner? no:
    # per-core in-edge counts c[i, n]: needs src owners, passed via closure
    raise NotImplementedError


def host_prep(x, src, dst, batch, W1, b1, W2, b2, W3, b3, n_graphs):
    import ml_dtypes

    N = x.shape[0]
    G = int(n_graphs)
    E = len(src)
    KW = 2
    CAP = KW * 128

    deg = np.bincount(dst, minlength=N).astype(np.float32)
    dis = np.where(deg > 0, 1.0 / np.sqrt(deg), 0.0).astype(np.float32)

    first = np.full(G, N, np.int64)
    np.minimum.at(first, batch.astype(np.int64), np.arange(N))

    # ---------- per-core destination bin-packing ----------
    # Node->core ownership stays contiguous (n // 2500). Within each core,
    # its 2500 destinations are packed into windows of <=128 nodes such
    # that each (src-core, window) cell holds at most CAP=KW*128 edges.
    NPC0 = N // N_CORES
    core_of = np.minimum(np.arange(N) // NPC0, N_CORES - 1)
    owner0 = core_of[src]
    cin = np.zeros((N_CORES, N), np.int32)
    np.add.at(cin, (owner0, dst), 1)

    vloc = np.full(N, -1, np.int64)
    slot_of = np.full(N, -1, np.int64)
    nwin_core = []
    for c in range(N_CORES):
        nodes = np.arange(c * NPC0, min((c + 1) * NPC0, N))
        order = nodes[np.argsort(-deg[nodes], kind="stable")]
        wins = []  # [count, percore_vec]
        open_w = []
        for n in order:
            cv = cin[:, n]
            placed = False
            for wi in open_w:
                cnt, vec = wins[wi]
                if cnt < 128 and np.all(vec + cv <= CAP):
                    vloc[n] = wi
                    slot_of[n] = cnt
                    wins[wi][0] += 1
                    vec += cv
                    placed = True
                    if wins[wi][0] == 128:
                        open_w.remove(wi)
                    break
            if not placed:
                wi = len(wins)
                wins.append([1, cv.astype(np.int64).copy()])
                vloc[n] = wi
                slot_of[n] = 0
                open_w.append(wi)
                if len(open_w) > 16:
                    open_w.pop(0)
        nwin_core.append(len(wins))

    wpc = max(nwin_core)
    qb = [round(k * wpc / QW) for k in range(QW + 1)]
    NPADc = wpc * 128
    lrow = vloc * 128 + slot_of  # local node row (window-major)

    def rmap(lr):
        return (lr % 128) * wpc + lr // 128

    # ---------- edges ----------
    owner_s = core_of[src]
    # processing order of local windows: quarter-major, then dst core, then v
    worder = []  # (core j, local v) -> position
    pos_of = np.full((N_CORES, wpc), -1, np.int64)
    p = 0
    for k in range(QW):
        for j in range(N_CORES):
            for v in range(qb[k], qb[k + 1]):
                pos_of[j, v] = p
                worder.append((j, v))
                p += 1
    NWIN_T = p  # == 8 * wpc
    NCH = NWIN_T * KW
    NSLOT = NCH * 128

    epos = pos_of[core_of[dst], vloc[dst]]  # per edge: window position

    # ---------- layer-3 edges ----------
    is_first = np.zeros(N, bool)
    is_first[first] = True
    gid_of = np.full(N, -1, np.int64)
    gid_of[first] = np.arange(G)
    e3 = np.nonzero(is_first[dst])[0]
    e3_owner = owner_s[e3]
    cnt3 = np.bincount(e3_owner, minlength=N_CORES)
    P3 = max(1, int(np.ceil(cnt3.max() / 128)))
    NS3 = P3 * 128

    eyeb = np.eye(128, dtype=ml_dtypes.bfloat16)
    eyef = np.eye(128, dtype=np.float32)
    b1c = np.ascontiguousarray(b1.reshape(2, 128).T)
    b2c = np.ascontiguousarray(b2.reshape(2, 128).T)
    b3p = np.zeros(ZPAD, np.float32)
    b3p[:C3] = b3
    b3bc = np.tile(b3p[None, :], (128, 1))
    disf = np.zeros((128, 1), np.float32)
    disf[:G, 0] = dis[first]

    w1b = W1.astype(ml_dtypes.bfloat16)
    w2r = np.zeros((128, 4 * 128), ml_dtypes.bfloat16)
    for kh in range(2):
        for mh in range(2):
            w2r[:, (kh * 2 + mh) * 128 : (kh * 2 + mh + 1) * 128] = W2[
                kh * 128 : (kh + 1) * 128, mh * 128 : (mh + 1) * 128
            ].astype(ml_dtypes.bfloat16)
    w3r = np.zeros((128, 2 * C3), ml_dtypes.bfloat16)
    for kh in range(2):
        w3r[:, kh * C3 : (kh + 1) * C3] = W3[kh * 128 : (kh + 1) * 128, :].astype(
            ml_dtypes.bfloat16
        )

    node_at = np.full((128, wpc, N_CORES), -1, np.int64)  # slot, v, core -> n
    node_at[slot_of, vloc, core_of] = np.arange(N)

    in_maps = []
    for i in range(N_CORES):
        sel = node_at[:, :, i]  # [128, wpc]
        valid = sel >= 0
        xs = np.zeros((128, wpc, C0), np.float32)
        xs[valid] = x[sel[valid]]
        dwin = np.zeros((128, wpc), np.float32)
        dwin[valid] = dis[sel[valid]]
        disw = np.ascontiguousarray(dwin)
        dl = np.zeros(NPADc, np.float32)
        dl[(vloc[sel[valid]] * 128 + slot_of[sel[valid]])] = dis[sel[valid]]
        disbc = np.tile(dl.astype(ml_dtypes.bfloat16)[None, :], (128, 1))

        ei = np.nonzero(owner_s == i)[0]
        ei = ei[np.argsort(epos[ei], kind="stable")]
        cntw = np.bincount(epos[ei], minlength=NWIN_T)
        assert cntw.max() <= CAP, f"bin packing failed: {cntw.max()}"
        ptr = np.concatenate([[0], np.cumsum(cntw)])
        eslot = np.zeros(len(ei), np.int64)
        for wp in range(NWIN_T):
            ee = np.arange(ptr[wp], ptr[wp + 1])
            eslot[ee] = wp * CAP + np.arange(len(ee))
        idx_flat = np.zeros(NSLOT, np.int64)
        idx_flat[eslot] = rmap(lrow[src[ei]])
        S_host = np.zeros((128, NCH, 128), ml_dtypes.float8_e4m3)
        S_host[eslot % 128, eslot // 128, slot_of[dst[ei]]] = 1.0
        idx_l = _pack_gather_idx(idx_flat, NSLOT)

        ee3 = e3[e3_owner == i]
        idx3 = _pack_gather_idx(rmap(lrow[src[ee3]]), NS3)
        S3_host = np.zeros((128, P3, 128), ml_dtypes.float8_e4m3)
        l3 = np.arange(len(ee3))
        S3_host[l3 % 128, l3 // 128, gid_of[dst[ee3]]] = 1.0

        in_maps.append(
            {
                "xs": xs,
                "idx": idx_l,
                "s": np.ascontiguousarray(S_host),
                "idx3": idx3,
                "s3": np.ascontiguousarray(S3_host),
                "disw": disw,
                "disbc": disbc,
                "disf": disf,
                "b1c": b1c,
                "b2c": b2c,
                "b3bc": b3bc,
                "w1": np.asarray(w1b),
                "w2r": np.asarray(w2r),
                "w3r": np.asarray(w3r),
                "eyeb": np.asarray(eyeb),
                "eyef": eyef,
            }
        )

    meta = dict(
        N=N, G=G, KW=KW, NCH=NCH, NSLOT=NSLOT, P3=P3, wpc=wpc, qb=tuple(qb)
    )
    return in_maps, meta


# ---------------------------------------------------------------------------
# Device program
# ---------------------------------------------------------------------------


def build_program(meta, compile_=True, repeat=1):
    import os

    V_NOGATHER = os.environ.get("V_NOGATHER") == "1"
    V_NOCOLL = os.environ.get("V_NOCOLL") == "1"
    V_NOMM = os.environ.get("V_NOMM") == "1"
    V_QN = int(os.environ.get("V_QN", "1"))  # swdge queues (round-robin)
    V_CALL = int(os.environ.get("V_CALL", "8"))  # gather chunks per call
    KW, NCH, NSLOT, P3 = meta["KW"], meta["NCH"], meta["NSLOT"], meta["P3"]
    wpc, qb = meta["wpc"], list(meta["qb"])
    qsz = [qb[k + 1] - qb[k] for k in range(QW)]
    NPADc = wpc * 128
    NS3 = P3 * 128
    CAP = KW * 128

    nc = bacc.Bacc(
        "TRN2", target_bir_lowering=False, debug=False, num_devices=N_CORES,
        num_swdge_queues=V_QN,
        dynamic_dma_scratch_size=max(16384, V_CALL * 128 * 16 * 2),
    )
    dp = nc.declare_dram_parameter
    xs_d = dp("xs", [128, wpc, C0], F32, isOutput=False)
    idx_d = dp("idx", [128, NSLOT // 16], I16, isOutput=False)
    s_d = dp("s", [128, NCH, 128], FP8, isOutput=False)
    idx3_d = dp("idx3", [128, NS3 // 16], I16, isOutput=False)
    s3_d = dp("s3", [128, P3, 128], FP8, isOutput=False)
    disw_d = dp("disw", [128, wpc], F32, isOutput=False)
    disbc_d = dp("disbc", [128, NPADc], BF16, isOutput=False)
    disf_d = dp("disf", [128, 1], F32, isOutput=False)
    b1c_d = dp("b1c", [128, 2], F32, isOutput=False)
    b2c_d = dp("b2c", [128, 2], F32, isOutput=False)
    b3bc_d = dp("b3bc", [128, ZPAD], F32, isOutput=False)
    w1_d = dp("w1", [128, C1], BF16, isOutput=False)
    w2r_d = dp("w2r", [128, 4 * 128], BF16, isOutput=False)
    w3r_d = dp("w3r", [128, 2 * C3], BF16, isOutput=False)
    eyeb_d = dp("eyeb", [128, 128], BF16, isOutput=False)
    eyef_d = dp("eyef", [128, 128], F32, isOutput=False)
    out_d = dp("out", [128, ZPAD], F32, isOutput=True)

    rg = [list(range(N_CORES))]
    AL = mybir.AluOpType
    ACT = mybir.ActivationFunctionType

    with tile.TileContext(nc) as tc:
        with (
            tc.tile_pool(name="const", bufs=1) as cpool,
            tc.tile_pool(name="work", bufs=4) as pool,
            tc.tile_pool(name="slab", bufs=1) as bigpool,
            tc.tile_pool(name="pslab", bufs=2) as spool,
            tc.tile_pool(name="gath", bufs=6) as gpool,
            tc.tile_pool(name="psA", bufs=2, space="PSUM") as psA,
            tc.tile_pool(name="psB", bufs=3, space="PSUM") as psB,
            tc.tile_pool(name="psD", bufs=2, space="PSUM") as psD,
            tc.tile_pool(name="psT", bufs=1, space="PSUM") as psT,
            tc.tile_pool(name="dram", bufs=1, space="DRAM") as dram,
        ):
            def cload(name, shape, dt, src, eng=None):
                t = cpool.tile(shape, dt, tag=name, name=name + "_sb")
                (eng or nc.sync).dma_start(out=t[:], in_=src)
                return t

            idx_sb = cload("idx", [128, NSLOT // 16], I16, idx_d[:, :])
            # S loaded in quarter chunks so early matmuls start sooner
            s_sb = cpool.tile([128, NCH, 128], FP8, tag="s", name="s_sb")
            SCH = [qb[k] * N_CORES * KW for k in range(QW + 1)]
            for k in range(QW):
                nc.sync.dma_start(
                    out=s_sb[:, SCH[k] : SCH[k + 1], :],
                    in_=s_d[:, SCH[k] : SCH[k + 1], :],
                )
            idx3_sb = cload("idx3", [128, NS3 // 16], I16, idx3_d[:, :])
            s3_sb = cload("s3", [128, P3, 128], FP8, s3_d[:, :, :])
            disw = cload("disw", [128, wpc], F32, disw_d[:, :])
            disbc = cload("disbc", [128, NPADc], BF16, disbc_d[:, :])
            disf = cload("disf", [128, 1], F32, disf_d[:, :])
            b1c = cload("b1c", [128, 2], F32, b1c_d[:, :])
            b2c = cload("b2c", [128, 2], F32, b2c_d[:, :])
            b3bc = cload("b3bc", [128, ZPAD], F32, b3bc_d[:, :])
            w1 = cload("w1", [128, C1], BF16, w1_d[:, :])
            w2r = cload("w2r", [128, 4 * 128], BF16, w2r_d[:, :])
            w3r = cload("w3r", [128, 2 * C3], BF16, w3r_d[:, :])
            eyeb = cload("eyeb", [128, 128], BF16, eyeb_d[:, :])
            eyef = cload("eyef", [128, 128], F32, eyef_d[:, :])

            for _rep in range(repeat):
                xt_loc = dram.tile([NPADc, C0], BF16)
                h1t_loc = dram.tile([NPADc, C1], BF16)
                zt_loc = dram.tile([NPADc, 128], BF16)
                p1q = [
                    dram.tile(
                        [N_CORES, C0, qsz[k] * 128], BF16, name=f"p1q{k}"
                    )
                    for k in range(QW)
                ]
                p2q = [
                    dram.tile(
                        [N_CORES, C1, qsz[k] * 128], BF16, name=f"p2q{k}"
                    )
                    for k in range(QW)
                ]
                a1q = [
                    dram.tile([C0, qsz[k] * 128], BF16, name=f"a1q{k}")
                    for k in range(QW)
                ]
                a2q = [
                    dram.tile([C1, qsz[k] * 128], BF16, name=f"a2q{k}")
                    for k in range(QW)
                ]
                ar_in = dram.tile([128, ZPAD], F32)
                ag3_d = dram.tile([N_CORES, 128, ZPAD], F32, addr_space="Shared")

                # ---- stage X: x-tilde table ----
                xsl = bigpool.tile([128, wpc, C0], F32, tag="xsl")
                nc.scalar.dma_start(out=xsl[:], in_=xs_d[:, :, :])
                xts = bigpool.tile([128, wpc, C0], BF16, tag="xts")
                for v in range(wpc):
                    nc.vector.tensor_scalar(
                        xts[:, v, :], xsl[:, v, :], disw[:, v : v + 1], None,
                        AL.mult,
                    )
                nc.sync.dma_start(out=xt_loc[:, :], in_=xts[:])

                # ---- partial aggregation (quarter-major) ----
                def agg_layer(table, Cin, pq, aq, tag):
                    CALL = V_CALL
                    ncalls = (NCH + CALL - 1) // CALL
                    gtiles = [None] * ncalls
                    issued = 0

                    def ensure(call_i):
                        nonlocal issued
                        while issued <= call_i:
                            m = issued
                            cs = min(CALL, NCH - m * CALL)
                            g = gpool.tile(
                                [128, CALL, Cin], BF16, tag=tag, name=tag + "g"
                            )
                            if V_NOGATHER:
                                nc.sync.dma_start(
                                    out=g[:, 0:cs, :],
                                    in_=table[0 : cs * 128, :],
                                )
                            else:
                                nc.gpsimd.dma_gather(
                                    g[:, 0:cs, :],
                                    table[:, :],
                                    idx_sb[:, m * CALL * 8 : (m * CALL + cs) * 8],
                                    num_idxs=cs * 128,
                                    num_idxs_reg=cs * 128,
                                    elem_size=Cin,
                                    queue_num=m % V_QN,
                                )
                            gtiles[m] = g
                            issued += 1

                    def emit_rs(k):
                        if not V_NOCOLL:
                            nc.gpsimd.collective_compute(
                                "ReduceScatter", AL.add, replica_groups=rg,
                                ins=[pq[k].opt()], outs=[aq[k].opt()],
                            )

                    nh = Cin // 128
                    wp = 0
                    for k in range(QW):
                        if k >= 2:
                            # two phases late: by now that phase's writes have
                            # completed, so the SEQ wait cannot stall gathers
                            emit_rs(k - 2)
                        for j in range(N_CORES):
                            pcs = [
                                spool.tile(
                                    [128, qsz[k] * 128], BF16,
                                    tag=f"{tag}s{h}", name=f"pc_{tag}{h}",
                                )
                                for h in range(nh)
                            ]
                            for t in range(qsz[k]):
                                ensure((wp * KW) // CALL)
                                ensure(((wp + 1) * KW - 1) // CALL)
                                ps = (psA if Cin == C0 else psB).tile(
                                    [128, Cin], F32, tag="agg"
                                )
                                for h in range(nh if not V_NOMM else 0):
                                    for jj in range(KW):
                                        kk = wp * KW + jj
                                        g = gtiles[kk // CALL]
                                        nc.tensor.matmul(
                                            ps[:, h * 128 : (h + 1) * 128],
                                            lhsT=g[
                                                :, kk % CALL,
                                                h * 128 : (h + 1) * 128,
                                            ],
                                            rhs=s_sb[:, kk, :],
                                            start=(jj == 0),
                                            stop=(jj == KW - 1),
                                        )
                                ws = t * 128
                                for h in range(nh):
                                    if Cin == C0 or h == 1:
                                        nc.scalar.activation(
                                            pcs[h][:, ws : ws + 128],
                                            ps[:, h * 128 : (h + 1) * 128],
                                            ACT.Copy,
                                        )
                                    else:
                                        nc.vector.tensor_copy(
                                            pcs[h][:, ws : ws + 128],
                                            ps[:, h * 128 : (h + 1) * 128],
                                        )
                                wp += 1
                            for h in range(nh):
                                nc.sync.dma_start(
                                    out=pq[k][j, h * 128 : (h + 1) * 128, :],
                                    in_=pcs[h][:],
                                )
                    for k in range(max(0, QW - 2), QW):
                        emit_rs(k)

                # ---- L1 ----
                agg_layer(xt_loc, C0, p1q, a1q, "g1")

                # ---- dense 1 ----
                h1slab = bigpool.tile([128, wpc, C1], BF16, tag="h1slab")
                for k in range(QW):
                    a1 = pool.tile(
                        [128, qsz[k] * 128], BF16, tag="a1", name="a1t"
                    )
                    nc.sync.dma_start(out=a1[:], in_=a1q[k][:, :])
                    for t in range(qsz[k]):
                        v = qb[k] + t
                        hp = psD.tile([128, C1], F32, tag="dense")
                        for mh in range(2):
                            nc.tensor.matmul(
                                hp[:, mh * 128 : (mh + 1) * 128],
                                lhsT=w1[:, mh * 128 : (mh + 1) * 128],
                                rhs=a1[:, t * 128 : (t + 1) * 128],
                                start=True,
                                stop=True,
                            )
                        dv = disbc[:, v * 128 : (v + 1) * 128]
                        for mh in range(2):
                            sl = slice(mh * 128, (mh + 1) * 128)
                            q = pool.tile([128, 128], F32, tag="q")
                            nc.vector.tensor_tensor(
                                q[:], hp[:, sl], dv, op=AL.mult
                            )
                            nc.vector.tensor_scalar(
                                q[:], q[:], b1c[:, mh : mh + 1], None, AL.add
                            )
                            vv = pool.tile([128, 128], F32, tag="v")
                            nc.scalar.activation(
                                vv[:], q[:], ACT.Copy, scale=0.01
                            )
                            nc.vector.tensor_tensor(q[:], q[:], vv[:], op=AL.max)
                            th = pool.tile([128, 128], BF16, tag="th")
                            nc.vector.tensor_tensor(th[:], q[:], dv, op=AL.mult)
                            tp = psT.tile([128, C1], BF16, tag="tr")
                            nc.tensor.transpose(tp[:, sl], th[:], eyeb[:])
                            nc.vector.tensor_copy(h1slab[:, v, sl], tp[:, sl])
                nc.sync.dma_start(out=h1t_loc[:, :], in_=h1slab[:])

                # ---- L2 ----
                agg_layer(h1t_loc, C1, p2q, a2q, "g2")

                # ---- dense 2 + z ----
                zslab = bigpool.tile([128, wpc, 128], BF16, tag="zslab")
                nc.vector.memset(zslab[:], 0.0)
                for k in range(QW):
                    a2 = [
                        pool.tile(
                            [128, qsz[k] * 128], BF16, tag=f"a2_{kh}",
                            name=f"a2t{kh}",
                        )
                        for kh in range(2)
                    ]
                    for kh in range(2):
                        nc.sync.dma_start(
                            out=a2[kh][:],
                            in_=a2q[k][kh * 128 : (kh + 1) * 128, :],
                        )
                    for t in range(qsz[k]):
                        v = qb[k] + t
                        hp = psD.tile([128, C2], F32, tag="dense")
                        for mh in range(2):
                            for kh in range(2):
                                nc.tensor.matmul(
                                    hp[:, mh * 128 : (mh + 1) * 128],
                                    lhsT=w2r[
                                        :,
                                        (kh * 2 + mh) * 128 : (kh * 2 + mh + 1)
                                        * 128,
                                    ],
                                    rhs=a2[kh][:, t * 128 : (t + 1) * 128],
                                    start=(kh == 0),
                                    stop=(kh == 1),
                                )
                        dv = disbc[:, v * 128 : (v + 1) * 128]
                        h2s = pool.tile([128, C2], BF16, tag="h2s")
                        for mh in range(2):
                            sl = slice(mh * 128, (mh + 1) * 128)
                            q = pool.tile([128, 128], F32, tag="q")
                            nc.vector.tensor_tensor(
                                q[:], hp[:, sl], dv, op=AL.mult
                            )
                            nc.vector.tensor_scalar(
                                q[:], q[:], b2c[:, mh : mh + 1], None, AL.add
                            )
                            vv = pool.tile([128, 128], F32, tag="v")
                            nc.scalar.activation(
                                vv[:], q[:], ACT.Copy, scale=0.01
                            )
                            nc.vector.tensor_tensor(q[:], q[:], vv[:], op=AL.max)
                            nc.vector.tensor_copy(h2s[:, sl], q[:])
                        zp = psD.tile([128, C2], F32, tag="dense")
                        for kh in range(2):
                            nc.tensor.matmul(
                                zp[0:C3, 0:128],
                                lhsT=w3r[:, kh * C3 : (kh + 1) * C3],
                                rhs=h2s[:, kh * 128 : (kh + 1) * 128],
                                start=(kh == 0),
                                stop=(kh == 1),
                            )
                        zs = pool.tile([128, 128], F32, tag="zs")
                        nc.vector.tensor_tensor(
                            zs[0:C3, :], zp[0:C3, 0:128], dv[0:C3, :], op=AL.mult
                        )
                        ztp = psD.tile([128, C2], F32, tag="dense")
                        nc.tensor.transpose(
                            ztp[:, 0:C3], zs[0:C3, :], eyef[0:C3, 0:C3]
                        )
                        nc.vector.tensor_copy(zslab[:, v, 0:C3], ztp[:, 0:C3])
                nc.sync.dma_start(out=zt_loc[:, :], in_=zslab[:])

                # ---- L3 readout ----
                g3 = gpool.tile([128, P3, 128], BF16, tag="g3")
                nc.gpsimd.dma_gather(
                    g3[:, :, :],
                    zt_loc[:, :],
                    idx3_sb[:, :],
                    num_idxs=NS3,
                    num_idxs_reg=NS3,
                    elem_size=128,
                )
                o3 = psD.tile([128, C2], F32, tag="dense")
                for pp in range(P3):
                    nc.tensor.matmul(
                        o3[:, 0:128],
                        lhsT=s3_sb[:, pp, :],
                        rhs=g3[:, pp, :],
                        start=(pp == 0),
                        stop=(pp == P3 - 1),
                    )
                o3s = pool.tile([128, ZPAD], F32, tag="o3s")
                nc.vector.tensor_copy(o3s[:], o3[:, 0:ZPAD])
                nc.sync.dma_start(out=ar_in[:, :], in_=o3s[:])
                nc.gpsimd.collective_compute(
                    "AllGather", AL.bypass, replica_groups=rg,
                    ins=[ar_in.opt()], outs=[ag3_d.opt()],
                )
                acc = pool.tile([128, ZPAD], F32, tag="acc")
                nc.sync.dma_start(out=acc[:], in_=ag3_d[0, :, :])
                for j in range(1, N_CORES):
                    t = pool.tile([128, ZPAD], F32, tag="accj", name="accj")
                    nc.sync.dma_start(out=t[:], in_=ag3_d[j, :, :])
                    nc.vector.tensor_tensor(acc[:], acc[:], t[:], op=AL.add)
                nc.vector.tensor_scalar(
                    acc[:], acc[:], disf[:, 0:1], None, AL.mult
                )
                nc.vector.tensor_tensor(acc[:], acc[:], b3bc[:], op=AL.add)
                nc.sync.dma_start(out=out_d[:, :], in_=acc[:])

    if compile_:
        nc.compile()
    return nc


# ---------------------------------------------------------------------------
# Entry point
# ---------------------------------------------------------------------------

_cache = {}


def _prepare(inputs):
    in_maps, meta = host_prep(**inputs)
    key = (meta["KW"], meta["NCH"], meta["P3"], meta["wpc"], meta["qb"])
    if key not in _cache:
        _cache[key] = build_program(meta)
    return _cache[key], in_maps, meta


def assemble_output(results, meta):
    G = meta["G"]
    return np.ascontiguousarray(results[0]["out"][:G, :C3])


def kernel(**inputs):
    nc, in_maps, meta = _prepare(inputs)
    res = run_bass_kernel_spmd(nc, in_maps, core_ids=list(range(N_CORES)))
    return assemble_output(res.results, meta)


if __name__ == "__main__":
    rng = np.random.default_rng(0)
    N, E, G = 20000, 320000, 100
    inputs = dict(
        x=rng.standard_normal((N, 128), dtype=np.float32),
        src=rng.integers(0, N, E).astype(np.int32),
        dst=rng.integers(0, N, E).astype(np.int32),
        batch=(np.arange(N) // (N // G)).astype(np.int32),
        W1=rng.standard_normal((128, 256), dtype=np.float32) / 11.3,
        b1=rng.standard_normal(256).astype(np.float32) * 0.01,
        W2=rng.standard_normal((256, 256), dtype=np.float32) / 16.0,
        b2=rng.standard_normal(256).astype(np.float32) * 0.01,
        W3=rng.standard_normal((256, 32), dtype=np.float32) / 16.0,
        b3=rng.standard_normal(32).astype(np.float32) * 0.01,
        n_graphs=G,
    )
    out = kernel(**inputs)
    print("out", out.shape, out.dtype, float(np.abs(out).max()))


# revision 50
# speedup vs baseline: 1.9751x; 1.0413x over previous
"""3-layer GCN (GCNConv x3 + leaky_relu + first-node-per-graph readout) on
8 Trainium2 NeuronCores via Bass/Tile.

Strategy (src-partitioned edges + quartered ReduceScatter):
  - Destination nodes are BIN-PACKED into windows of <=128 nodes such that
    every (src-core, window) cell holds at most KW*128 = 256 edges; windows
    are assigned contiguously to 8 cores. Each core owns the feature rows
    (x-tilde / h1-tilde tables in local DRAM) of its windows' nodes, and all
    edges whose SOURCE it owns -- gathers only ever touch local tables, so
    no feature AllGather exists.
  - GCN normalization is factored: out = dis_d * segsum_dst((dis*h)[src]) @ W
    + b with dis = deg^-1/2.
  - Per edge-chunk of 128, one PE matmul psum[c, d] += g[e, c]^T @ S[e, d]
    with S a host-precomputed fp8 one-hot (edge -> dst slot) matrix builds
    CHANNEL-MAJOR partial aggregations for all windows.
  - Windows are processed in two phases (phase-major, then dst-core, then
    window); after each phase a ReduceScatter (op=add) reduces that phase's
    partials, overlapping the first collective with later aggregation.
  - Layer 3 only needs ~1.6k edges (dst == first node of a graph): each core
    aggregates its local z-tilde rows into per-graph partial sums [128, 64];
    the host sums the 8 partial outputs and applies dis[first]/b3.

kernel(**inputs) takes the full unsharded inputs and returns the full
[n_graphs, 32] float32 output.
"""

import sys

sys.path.insert(0, "/opt/trn_rl_repo")

import numpy as np

import concourse.bacc as bacc
import concourse.mybir as mybir
import concourse.tile as tile
from concourse.bass_utils import run_bass_kernel_spmd

F32 = mybir.dt.float32
BF16 = mybir.dt.bfloat16
FP8 = mybir.dt.float8e4
I16 = mybir.dt.int16

N_CORES = 8
C0, C1, C2, C3 = 128, 256, 256, 32
ZPAD = 64
QW = 2  # reduce-scatter phases

OUT_NAMES = ["out"]


def _pack_gather_idx(idx, n_slots):
    """int32 row indices -> dma_gather int16 layout [128, n_slots//16]."""
    assert n_slots % 16 == 0
    a = np.zeros(n_slots, np.int16)
    a[: len(idx)] = idx.astype(np.int16)
    a = a.reshape(n_slots // 16, 16).T  # [16, cols]
    return np.tile(a, (8, 1))  # [128, cols]


def host_prep(x, src, dst, batch, W1, b1, W2, b2, W3, b3, n_graphs):
    import ml_dtypes

    N = x.shape[0]
    G = int(n_graphs)
    E = len(src)
    KW = 2
    CAP = KW * 128

    deg = np.bincount(dst, minlength=N).astype(np.float32)
    dis = np.where(deg > 0, 1.0 / np.sqrt(deg), 0.0).astype(np.float32)

    first = np.full(G, N, np.int64)
    np.minimum.at(first, batch.astype(np.int64), np.arange(N))

    # ---------- per-core destination bin-packing ----------
    # Node->core ownership stays contiguous (n // 2500). Within each core,
    # its 2500 destinations are packed into windows of <=128 nodes such
    # that each (src-core, window) cell holds at most CAP=KW*128 edges.
    NPC0 = N // N_CORES
    core_of = np.minimum(np.arange(N) // NPC0, N_CORES - 1)
    owner0 = core_of[src]
    cin = np.zeros((N_CORES, N), np.int32)
    np.add.at(cin, (owner0, dst), 1)

    vloc = np.full(N, -1, np.int64)
    slot_of = np.full(N, -1, np.int64)
    nwin_core = []
    for c in range(N_CORES):
        nodes = np.arange(c * NPC0, min((c + 1) * NPC0, N))
        order = nodes[np.argsort(-deg[nodes], kind="stable")]
        wins = []  # [count, percore_vec]
        open_w = []
        for n in order:
            cv = cin[:, n]
            placed = False
            for wi in open_w:
                cnt, vec = wins[wi]
                if cnt < 128 and np.all(vec + cv <= CAP):
                    vloc[n] = wi
                    slot_of[n] = cnt
                    wins[wi][0] += 1
                    vec += cv
                    placed = True
                    if wins[wi][0] == 128:
                        open_w.remove(wi)
                    break
            if not placed:
                wi = len(wins)
                wins.append([1, cv.astype(np.int64).copy()])
                vloc[n] = wi
                slot_of[n] = 0
                open_w.append(wi)
                if len(open_w) > 16:
                    open_w.pop(0)
        nwin_core.append(len(wins))

    wpc = max(nwin_core)
    qb = [round(k * wpc / QW) for k in range(QW + 1)]
    NPADc = wpc * 128
    lrow = vloc * 128 + slot_of  # local node row (window-major)

    def rmap(lr):
        return (lr % 128) * wpc + lr // 128

    # ---------- edges ----------
    owner_s = core_of[src]
    # processing order of local windows: quarter-major, then dst core, then v
    worder = []  # (core j, local v) -> position
    pos_of = np.full((N_CORES, wpc), -1, np.int64)
    p = 0
    for k in range(QW):
        for j in range(N_CORES):
            for v in range(qb[k], qb[k + 1]):
                pos_of[j, v] = p
                worder.append((j, v))
                p += 1
    NWIN_T = p  # == 8 * wpc
    NCH = NWIN_T * KW
    NSLOT = NCH * 128

    epos = pos_of[core_of[dst], vloc[dst]]  # per edge: window position

    # ---------- layer-3 edges ----------
    is_first = np.zeros(N, bool)
    is_first[first] = True
    gid_of = np.full(N, -1, np.int64)
    gid_of[first] = np.arange(G)
    e3 = np.nonzero(is_first[dst])[0]
    e3_owner = owner_s[e3]
    cnt3 = np.bincount(e3_owner, minlength=N_CORES)
    P3 = max(1, int(np.ceil(cnt3.max() / 128)))
    NS3 = P3 * 128

    eyeb = np.eye(128, dtype=ml_dtypes.bfloat16)
    eyef = np.eye(128, dtype=np.float32)
    b1c = np.ascontiguousarray(b1.reshape(2, 128).T)
    b2c = np.ascontiguousarray(b2.reshape(2, 128).T)
    b3p = np.zeros(ZPAD, np.float32)
    b3p[:C3] = b3
    b3bc = np.tile(b3p[None, :], (128, 1))
    disf = np.zeros((128, 1), np.float32)
    disf[:G, 0] = dis[first]

    w1b = W1.astype(ml_dtypes.bfloat16)
    w2r = np.zeros((128, 4 * 128), ml_dtypes.bfloat16)
    for kh in range(2):
        for mh in range(2):
            w2r[:, (kh * 2 + mh) * 128 : (kh * 2 + mh + 1) * 128] = W2[
                kh * 128 : (kh + 1) * 128, mh * 128 : (mh + 1) * 128
            ].astype(ml_dtypes.bfloat16)
    w3r = np.zeros((128, 2 * C3), ml_dtypes.bfloat16)
    for kh in range(2):
        w3r[:, kh * C3 : (kh + 1) * C3] = W3[kh * 128 : (kh + 1) * 128, :].astype(
            ml_dtypes.bfloat16
        )

    node_at = np.full((128, wpc, N_CORES), -1, np.int64)  # slot, v, core -> n
    node_at[slot_of, vloc, core_of] = np.arange(N)

    in_maps = []
    for i in range(N_CORES):
        sel = node_at[:, :, i]  # [128, wpc]
        valid = sel >= 0
        xs = np.zeros((128, wpc, C0), np.float32)
        xs[valid] = x[sel[valid]]
        dwin = np.zeros((128, wpc), np.float32)
        dwin[valid] = dis[sel[valid]]
        disw = np.ascontiguousarray(dwin)
        dl = np.zeros(NPADc, np.float32)
        dl[(vloc[sel[valid]] * 128 + slot_of[sel[valid]])] = dis[sel[valid]]
        disbc = np.tile(dl.astype(ml_dtypes.bfloat16)[None, :], (128, 1))

        ei = np.nonzero(owner_s == i)[0]
        ei = ei[np.argsort(epos[ei], kind="stable")]
        cntw = np.bincount(epos[ei], minlength=NWIN_T)
        assert cntw.max() <= CAP, f"bin packing failed: {cntw.max()}"
        ptr = np.concatenate([[0], np.cumsum(cntw)])
        eslot = np.zeros(len(ei), np.int64)
        for wp in range(NWIN_T):
            ee = np.arange(ptr[wp], ptr[wp + 1])
            eslot[ee] = wp * CAP + np.arange(len(ee))
        idx_flat = np.zeros(NSLOT, np.int64)
        idx_flat[eslot] = rmap(lrow[src[ei]])
        S_host = np.zeros((128, NCH, 128), ml_dtypes.float8_e4m3)
        S_host[eslot % 128, eslot // 128, slot_of[dst[ei]]] = 1.0
        idx_l = _pack_gather_idx(idx_flat, NSLOT)

        ee3 = e3[e3_owner == i]
        idx3 = _pack_gather_idx(rmap(lrow[src[ee3]]), NS3)
        S3_host = np.zeros((128, P3, 128), ml_dtypes.float8_e4m3)
        l3 = np.arange(len(ee3))
        S3_host[l3 % 128, l3 // 128, gid_of[dst[ee3]]] = 1.0

        in_maps.append(
            {
                "xs": xs,
                "idx": idx_l,
                "s": np.ascontiguousarray(S_host),
                "idx3": idx3,
                "s3": np.ascontiguousarray(S3_host),
                "disw": disw,
                "disbc": disbc,
                "disf": disf,
                "b1c": b1c,
                "b2c": b2c,
                "b3bc": b3bc,
                "w1": np.asarray(w1b),
                "w2r": np.asarray(w2r),
                "w3r": np.asarray(w3r),
                "eyeb": np.asarray(eyeb),
                "eyef": eyef,
            }
        )

    meta = dict(
        N=N, G=G, KW=KW, NCH=NCH, NSLOT=NSLOT, P3=P3, wpc=wpc, qb=tuple(qb),
        disf=disf[:, 0:1].copy(), b3=b3.copy(),
    )
    return in_maps, meta


# ---------------------------------------------------------------------------
# Device program
# ---------------------------------------------------------------------------


def build_program(meta, compile_=True, repeat=1):
    import os

    V_NOGATHER = os.environ.get("V_NOGATHER") == "1"
    V_NOCOLL = os.environ.get("V_NOCOLL") == "1"
    V_NOMM = os.environ.get("V_NOMM") == "1"
    V_QN = int(os.environ.get("V_QN", "4"))  # swdge queues (round-robin)
    V_CALL = int(os.environ.get("V_CALL", "8"))  # gather chunks per call
    KW, NCH, NSLOT, P3 = meta["KW"], meta["NCH"], meta["NSLOT"], meta["P3"]
    wpc, qb = meta["wpc"], list(meta["qb"])
    qsz = [qb[k + 1] - qb[k] for k in range(QW)]
    NPADc = wpc * 128
    NS3 = P3 * 128
    CAP = KW * 128

    nc = bacc.Bacc(
        "TRN2", target_bir_lowering=False, debug=False, num_devices=N_CORES,
        num_swdge_queues=V_QN,
        dynamic_dma_scratch_size=max(16384, V_CALL * 128 * 16 * 2),
    )
    dp = nc.declare_dram_parameter
    xs_d = dp("xs", [128, wpc, C0], F32, isOutput=False)
    idx_d = dp("idx", [128, NSLOT // 16], I16, isOutput=False)
    s_d = dp("s", [128, NCH, 128], FP8, isOutput=False)
    idx3_d = dp("idx3", [128, NS3 // 16], I16, isOutput=False)
    s3_d = dp("s3", [128, P3, 128], FP8, isOutput=False)
    disw_d = dp("disw", [128, wpc], F32, isOutput=False)
    disbc_d = dp("disbc", [128, NPADc], BF16, isOutput=False)
    disf_d = dp("disf", [128, 1], F32, isOutput=False)
    b1c_d = dp("b1c", [128, 2], F32, isOutput=False)
    b2c_d = dp("b2c", [128, 2], F32, isOutput=False)
    b3bc_d = dp("b3bc", [128, ZPAD], F32, isOutput=False)
    w1_d = dp("w1", [128, C1], BF16, isOutput=False)
    w2r_d = dp("w2r", [128, 4 * 128], BF16, isOutput=False)
    w3r_d = dp("w3r", [128, 2 * C3], BF16, isOutput=False)
    eyeb_d = dp("eyeb", [128, 128], BF16, isOutput=False)
    eyef_d = dp("eyef", [128, 128], F32, isOutput=False)
    out_d = dp("out", [128, ZPAD], F32, isOutput=True)

    rg = [list(range(N_CORES))]
    AL = mybir.AluOpType
    ACT = mybir.ActivationFunctionType

    with tile.TileContext(nc) as tc:
        with (
            tc.tile_pool(name="const", bufs=1) as cpool,
            tc.tile_pool(name="work", bufs=4) as pool,
            tc.tile_pool(name="slab", bufs=1) as bigpool,
            tc.tile_pool(name="pslab", bufs=2) as spool,
            tc.tile_pool(name="gath", bufs=5) as gpool,
            tc.tile_pool(name="psA", bufs=2, space="PSUM") as psA,
            tc.tile_pool(name="psB", bufs=3, space="PSUM") as psB,
            tc.tile_pool(name="psD", bufs=2, space="PSUM") as psD,
            tc.tile_pool(name="psT", bufs=1, space="PSUM") as psT,
            tc.tile_pool(name="dram", bufs=1, space="DRAM") as dram,
        ):
            def cload(name, shape, dt, src, eng=None):
                t = cpool.tile(shape, dt, tag=name, name=name + "_sb")
                (eng or nc.sync).dma_start(out=t[:], in_=src)
                return t

            idx_sb = cload("idx", [128, NSLOT // 16], I16, idx_d[:, :])
            # S loaded in quarter chunks so early matmuls start sooner
            s_sb = cpool.tile([128, NCH, 128], FP8, tag="s", name="s_sb")
            SCH = [qb[k] * N_CORES * KW for k in range(QW + 1)]
            for k in range(QW):
                nc.sync.dma_start(
                    out=s_sb[:, SCH[k] : SCH[k + 1], :],
                    in_=s_d[:, SCH[k] : SCH[k + 1], :],
                )
            idx3_sb = cload("idx3", [128, NS3 // 16], I16, idx3_d[:, :])
            s3_sb = cload("s3", [128, P3, 128], FP8, s3_d[:, :, :])
            disw = cload("disw", [128, wpc], F32, disw_d[:, :])
            disbc = cload("disbc", [128, NPADc], BF16, disbc_d[:, :])
            disf = cload("disf", [128, 1], F32, disf_d[:, :])
            b1c = cload("b1c", [128, 2], F32, b1c_d[:, :])
            b2c = cload("b2c", [128, 2], F32, b2c_d[:, :])
            b3bc = cload("b3bc", [128, ZPAD], F32, b3bc_d[:, :])
            w1 = cload("w1", [128, C1], BF16, w1_d[:, :])
            w2r = cload("w2r", [128, 4 * 128], BF16, w2r_d[:, :])
            w3r = cload("w3r", [128, 2 * C3], BF16, w3r_d[:, :])
            eyeb = cload("eyeb", [128, 128], BF16, eyeb_d[:, :])
            eyef = cload("eyef", [128, 128], F32, eyef_d[:, :])

            for _rep in range(repeat):
                xt_loc = dram.tile([NPADc, C0], BF16)
                h1t_loc = dram.tile([NPADc, C1], BF16)
                zt_loc = dram.tile([NPADc, 128], BF16)
                p1q = [
                    dram.tile(
                        [N_CORES, C0, qsz[k] * 128], BF16, name=f"p1q{k}"
                    )
                    for k in range(QW)
                ]
                p2q = [
                    dram.tile(
                        [N_CORES, C1, qsz[k] * 128], BF16, name=f"p2q{k}"
                    )
                    for k in range(QW)
                ]
                a1q = [
                    dram.tile([C0, qsz[k] * 128], BF16, name=f"a1q{k}")
                    for k in range(QW)
                ]
                a2q = [
                    dram.tile([C1, qsz[k] * 128], BF16, name=f"a2q{k}")
                    for k in range(QW)
                ]

                # ---- stage X: x-tilde table ----
                xsl = bigpool.tile([128, wpc, C0], F32, tag="xsl")
                nc.scalar.dma_start(out=xsl[:], in_=xs_d[:, :, :])
                xts = bigpool.tile([128, wpc, C0], BF16, tag="xts")
                for v in range(wpc):
                    nc.vector.tensor_scalar(
                        xts[:, v, :], xsl[:, v, :], disw[:, v : v + 1], None,
                        AL.mult,
                    )
                nc.sync.dma_start(out=xt_loc[:, :], in_=xts[:])

                # ---- partial aggregation (quarter-major) ----
                def agg_layer(table, Cin, pq, aq, tag):
                    CALL = V_CALL
                    ncalls = (NCH + CALL - 1) // CALL
                    gtiles = [None] * ncalls
                    issued = 0

                    def ensure(call_i):
                        nonlocal issued
                        while issued <= call_i:
                            m = issued
                            cs = min(CALL, NCH - m * CALL)
                            g = gpool.tile(
                                [128, CALL, Cin], BF16, tag=tag, name=tag + "g"
                            )
                            if V_NOGATHER:
                                nc.sync.dma_start(
                                    out=g[:, 0:cs, :],
                                    in_=table[0 : cs * 128, :],
                                )
                            else:
                                nc.gpsimd.dma_gather(
                                    g[:, 0:cs, :],
                                    table[:, :],
                                    idx_sb[:, m * CALL * 8 : (m * CALL + cs) * 8],
                                    num_idxs=cs * 128,
                                    num_idxs_reg=cs * 128,
                                    elem_size=Cin,
                                    queue_num=m % V_QN,
                                )
                            gtiles[m] = g
                            issued += 1

                    def emit_rs(k):
                        if not V_NOCOLL:
                            nc.gpsimd.collective_compute(
                                "ReduceScatter", AL.add, replica_groups=rg,
                                ins=[pq[k].opt()], outs=[aq[k].opt()],
                            )

                    nh = Cin // 128
                    wp = 0
                    for k in range(QW):
                        for j in range(N_CORES):
                            pcs = [
                                spool.tile(
                                    [128, qsz[k] * 128], BF16,
                                    tag=f"{tag}s{h}", name=f"pc_{tag}{h}",
                                )
                                for h in range(nh)
                            ]
                            for t in range(qsz[k]):
                                ensure((wp * KW) // CALL)
                                ensure(((wp + 1) * KW - 1) // CALL)
                                ps = (psA if Cin == C0 else psB).tile(
                                    [128, Cin], F32, tag="agg"
                                )
                                for h in range(nh if not V_NOMM else 0):
                                    for jj in range(KW):
                                        kk = wp * KW + jj
                                        g = gtiles[kk // CALL]
                                        nc.tensor.matmul(
                                            ps[:, h * 128 : (h + 1) * 128],
                                            lhsT=g[
                                                :, kk % CALL,
                                                h * 128 : (h + 1) * 128,
                                            ],
                                            rhs=s_sb[:, kk, :],
                                            start=(jj == 0),
                                            stop=(jj == KW - 1),
                                        )
                                ws = t * 128
                                for h in range(nh):
                                    if Cin == C0 or h == 1:
                                        nc.scalar.activation(
                                            pcs[h][:, ws : ws + 128],
                                            ps[:, h * 128 : (h + 1) * 128],
                                            ACT.Copy,
                                        )
                                    else:
                                        nc.vector.tensor_copy(
                                            pcs[h][:, ws : ws + 128],
                                            ps[:, h * 128 : (h + 1) * 128],
                                        )
                                wp += 1
                            for h in range(nh):
                                nc.sync.dma_start(
                                    out=pq[k][j, h * 128 : (h + 1) * 128, :],
                                    in_=pcs[h][:],
                                )
                        emit_rs(k)

                # ---- L1 ----
                agg_layer(xt_loc, C0, p1q, a1q, "g1")

                # ---- dense 1 ----
                h1slab = bigpool.tile([128, wpc, C1], BF16, tag="h1slab")
                for k in range(QW):
                    a1 = pool.tile(
                        [128, qsz[k] * 128], BF16, tag="a1", name="a1t"
                    )
                    nc.sync.dma_start(out=a1[:], in_=a1q[k][:, :])
                    for t in range(qsz[k]):
                        v = qb[k] + t
                        hp = psD.tile([128, C1], F32, tag="dense")
                        for mh in range(2):
                            nc.tensor.matmul(
                                hp[:, mh * 128 : (mh + 1) * 128],
                                lhsT=w1[:, mh * 128 : (mh + 1) * 128],
                                rhs=a1[:, t * 128 : (t + 1) * 128],
                                start=True,
                                stop=True,
                            )
                        dv = disbc[:, v * 128 : (v + 1) * 128]
                        for mh in range(2):
                            sl = slice(mh * 128, (mh + 1) * 128)
                            q = pool.tile([128, 128], F32, tag="q")
                            nc.vector.tensor_tensor(
                                q[:], hp[:, sl], dv, op=AL.mult
                            )
                            nc.vector.tensor_scalar(
                                q[:], q[:], b1c[:, mh : mh + 1], None, AL.add
                            )
                            vv = pool.tile([128, 128], F32, tag="v")
                            nc.scalar.activation(
                                vv[:], q[:], ACT.Copy, scale=0.01
                            )
                            nc.vector.tensor_tensor(q[:], q[:], vv[:], op=AL.max)
                            th = pool.tile([128, 128], BF16, tag="th")
                            nc.vector.tensor_tensor(th[:], q[:], dv, op=AL.mult)
                            tp = psT.tile([128, C1], BF16, tag="tr")
                            nc.tensor.transpose(tp[:, sl], th[:], eyeb[:])
                            nc.vector.tensor_copy(h1slab[:, v, sl], tp[:, sl])
                nc.sync.dma_start(out=h1t_loc[:, :], in_=h1slab[:])

                # ---- L2 ----
                agg_layer(h1t_loc, C1, p2q, a2q, "g2")

                # ---- dense 2 + z ----
                zslab = bigpool.tile([128, wpc, 128], BF16, tag="zslab")
                nc.vector.memset(zslab[:], 0.0)
                for k in range(QW):
                    a2 = [
                        pool.tile(
                            [128, qsz[k] * 128], BF16, tag=f"a2_{kh}",
                            name=f"a2t{kh}",
                        )
                        for kh in range(2)
                    ]
                    for kh in range(2):
                        nc.sync.dma_start(
                            out=a2[kh][:],
                            in_=a2q[k][kh * 128 : (kh + 1) * 128, :],
                        )
                    for t in range(qsz[k]):
                        v = qb[k] + t
                        hp = psD.tile([128, C2], F32, tag="dense")
                        for mh in range(2):
                            for kh in range(2):
                                nc.tensor.matmul(
                                    hp[:, mh * 128 : (mh + 1) * 128],
                                    lhsT=w2r[
                                        :,
                                        (kh * 2 + mh) * 128 : (kh * 2 + mh + 1)
                                        * 128,
                                    ],
                                    rhs=a2[kh][:, t * 128 : (t + 1) * 128],
                                    start=(kh == 0),
                                    stop=(kh == 1),
                                )
                        dv = disbc[:, v * 128 : (v + 1) * 128]
                        h2s = pool.tile([128, C2], BF16, tag="h2s")
                        for mh in range(2):
                            sl = slice(mh * 128, (mh + 1) * 128)
                            q = pool.tile([128, 128], F32, tag="q")
                            nc.vector.tensor_tensor(
                                q[:], hp[:, sl], dv, op=AL.mult
                            )
                            nc.vector.tensor_scalar(
                                q[:], q[:], b2c[:, mh : mh + 1], None, AL.add
                            )
                            vv = pool.tile([128, 128], F32, tag="v")
                            nc.scalar.activation(
                                vv[:], q[:], ACT.Copy, scale=0.01
                            )
                            nc.vector.tensor_tensor(q[:], q[:], vv[:], op=AL.max)
                            nc.vector.tensor_copy(h2s[:, sl], q[:])
                        zp = psD.tile([128, C2], F32, tag="dense")
                        for kh in range(2):
                            nc.tensor.matmul(
                                zp[0:C3, 0:128],
                                lhsT=w3r[:, kh * C3 : (kh + 1) * C3],
                                rhs=h2s[:, kh * 128 : (kh + 1) * 128],
                                start=(kh == 0),
                                stop=(kh == 1),
                            )
                        zs = pool.tile([128, 128], F32, tag="zs")
                        nc.vector.tensor_tensor(
                            zs[0:C3, :], zp[0:C3, 0:128], dv[0:C3, :], op=AL.mult
                        )
                        ztp = psD.tile([128, C2], F32, tag="dense")
                        nc.tensor.transpose(
                            ztp[:, 0:C3], zs[0:C3, :], eyef[0:C3, 0:C3]
                        )
                        nc.vector.tensor_copy(zslab[:, v, 0:C3], ztp[:, 0:C3])
                nc.sync.dma_start(out=zt_loc[:, :], in_=zslab[:])

                # ---- L3 readout ----
                g3 = gpool.tile([128, P3, 128], BF16, tag="g3")
                nc.gpsimd.dma_gather(
                    g3[:, :, :],
                    zt_loc[:, :],
                    idx3_sb[:, :],
                    num_idxs=NS3,
                    num_idxs_reg=NS3,
                    elem_size=128,
                )
                o3 = psD.tile([128, C2], F32, tag="dense")
                for pp in range(P3):
                    nc.tensor.matmul(
                        o3[:, 0:128],
                        lhsT=s3_sb[:, pp, :],
                        rhs=g3[:, pp, :],
                        start=(pp == 0),
                        stop=(pp == P3 - 1),
                    )
                o3s = pool.tile([128, ZPAD], F32, tag="o3s")
                nc.vector.tensor_copy(o3s[:], o3[:, 0:ZPAD])
                nc.sync.dma_start(out=out_d[:, :], in_=o3s[:])

    if compile_:
        nc.compile()
    return nc


# ---------------------------------------------------------------------------
# Entry point
# ---------------------------------------------------------------------------

_cache = {}


def _prepare(inputs):
    in_maps, meta = host_prep(**inputs)
    key = (meta["KW"], meta["NCH"], meta["P3"], meta["wpc"], meta["qb"])
    if key not in _cache:
        _cache[key] = build_program(meta)
    return _cache[key], in_maps, meta


def assemble_output(results, meta):
    G = meta["G"]
    tot = np.zeros((128, ZPAD), np.float32)
    for r in results:
        tot += r["out"]
    out = tot[:G, :C3] * meta["disf"][:G] + meta["b3"][None, :]
    return np.ascontiguousarray(out.astype(np.float32))


def kernel(**inputs):
    nc, in_maps, meta = _prepare(inputs)
    res = run_bass_kernel_spmd(nc, in_maps, core_ids=list(range(N_CORES)))
    return assemble_output(res.results, meta)


if __name__ == "__main__":
    rng = np.random.default_rng(0)
    N, E, G = 20000, 320000, 100
    inputs = dict(
        x=rng.standard_normal((N, 128), dtype=np.float32),
        src=rng.integers(0, N, E).astype(np.int32),
        dst=rng.integers(0, N, E).astype(np.int32),
        batch=(np.arange(N) // (N // G)).astype(np.int32),
        W1=rng.standard_normal((128, 256), dtype=np.float32) / 11.3,
        b1=rng.standard_normal(256).astype(np.float32) * 0.01,
        W2=rng.standard_normal((256, 256), dtype=np.float32) / 16.0,
        b2=rng.standard_normal(256).astype(np.float32) * 0.01,
        W3=rng.standard_normal((256, 32), dtype=np.float32) / 16.0,
        b3=rng.standard_normal(32).astype(np.float32) * 0.01,
        n_graphs=G,
    )
    out = kernel(**inputs)
    print("out", out.shape, out.dtype, float(np.abs(out).max()))
